# revision 1
# baseline (speedup 1.0000x reference)
"""Trainium2 Bass kernel for CAConv2 (coordinate-attention + 3x3 conv block).

Shapes (hardcoded): x (8, 128, 128, 128) f32; data-parallel over batch,
one image per NeuronCore (8 cores).

Scheduling facts learned from HW traces / the cost model:
- PE p-states 0.65/1.2/2.4 GHz; full clock needs ~3us of continuous PE
  busy, and multi-us idle drops it again. Warm matmuls bridge the gaps.
- Engines list-schedule: ready instructions run in emission order, so
  emission position is priority, dependencies are eligibility.
- Cross-engine dependency hops cost ~0.5-1.3us of semaphore latency.
- x streams at ~366 GB/s (~11.5us); a_w needs all of x, so the conv
  cannot start before ~x_done + the a_w chain (~5us).
- DVE is the scarce phase-1 resource: the x_h pooling for rows 0-63 is
  done on the PE instead (range-prescaled w1 seg-matmuls + one psum
  reduce); rows 64-127 use DVE add-trees during the late stream window.
"""

import numpy as np
import ml_dtypes

import concourse.bacc as bacc
import concourse.tile as tile
from concourse import mybir
from concourse.bass import ds
from concourse.bass_utils import run_bass_kernel_spmd

BF16 = mybir.dt.bfloat16
F32 = mybir.dt.float32
C, H, W, MIP = 128, 128, 128, 8
WP = W + 4  # padded width: cols [2, 130) hold data, 0/1 and 130/131 are zero
HP = H + 2  # padded height: rows [1, 129) hold data
EPS = 1e-5
AF = mybir.ActivationFunctionType
ALU = mybir.AluOpType

_CACHE = {}


def build_nc():
    nc = bacc.Bacc(num_swdge_queues=2)
    xp = nc.declare_dram_parameter("x", [C, H * W], BF16, isOutput=False)
    w1ts = nc.declare_dram_parameter("w1ts", [C, 3 * MIP], BF16, isOutput=False)
    wht = nc.declare_dram_parameter("wht", [MIP, C], BF16, isOutput=False)
    wwt = nc.declare_dram_parameter("wwt", [MIP, C], BF16, isOutput=False)
    # wct[i, k, o] = wc[o, i, k//3, k%3]
    wct = nc.declare_dram_parameter("wct", [C, 9 * C], BF16, isOutput=False)
    # p8 cols: 0: s1/6, 1: t1f/6, 2: s1, 3: t1f+3   (t1f = s1*b1 + be1 - m1*s1)
    p8 = nc.declare_dram_parameter("p8", [MIP, 4], F32, isOutput=False)
    # p128 cols: 0: bh, 1: bw, 2: s2, 3: b2 (= bc*s2 + be2 - m2*s2)
    p128 = nc.declare_dram_parameter("p128", [C, 4], F32, isOutput=False)
    outp = nc.declare_dram_parameter("out", [C, H, W], BF16, isOutput=True)

    c1, c2, c3 = 7.0 / 128, 3.0 / 128, 1.0 / 128

    with tile.TileContext(nc) as tc:
        with (
            tc.tile_pool(name="sing", bufs=1) as sing,
            tc.tile_pool(name="pp", bufs=2) as pp,
            tc.tile_pool(name="small", bufs=1) as small,
        ):
            xs = sing.tile([C, H * W], BF16)
            ug = sing.tile([C, HP, WP], BF16)
            s32b = sing.tile([C, 64, 4], F32)  # col-segment sums rows 64-127
            wtile = sing.tile([C, 512], BF16)  # zeros; warm-matmul fodder

            # weights/params ride the sync ring; w1ts first (first matmuls)
            w1ts_sb = sing.tile([C, 3, MIP], BF16)
            nc.sync.dma_start(
                out=w1ts_sb, in_=w1ts.rearrange("c (r m) -> c r m", r=3)
            )
            # x chunks all on the gpsimd SWDGE ring: descriptors drain in
            # issue order -> staggered completion at full BW.
            XCH = [(0, 16), (16, 16), (32, 32), (64, 32), (96, 16), (112, 8), (120, 8)]
            for r0, nr in XCH:
                nc.gpsimd.dma_start(
                    out=xs[:, ds(r0 * W, nr * W)],
                    in_=xp[:, ds(r0 * W, nr * W)],
                )
            wht_sb = sing.tile([MIP, C], BF16)
            nc.sync.dma_start(out=wht_sb, in_=wht[:, :])
            wwt_sb = sing.tile([MIP, C], BF16)
            nc.sync.dma_start(out=wwt_sb, in_=wwt[:, :])
            p8_sb = sing.tile([MIP, 4], F32)
            nc.sync.dma_start(out=p8_sb, in_=p8[:, :])
            p128_sb = sing.tile([C, 4], F32)
            nc.sync.dma_start(out=p128_sb, in_=p128[:, :])
            wct_sb = sing.tile([C, 9, C], BF16)
            nc.sync.dma_start(out=wct_sb, in_=wct.rearrange("i (k o) -> i k o", k=9))

            # warm fodder first so the PE can start ramping ASAP
            nc.vector.memset(wtile, 0.0)
            # conv padding border of ug
            nc.vector.memset(ug[:, 0, :], 0.0)
            nc.vector.memset(ug[:, HP - 1, :], 0.0)
            nc.vector.memset(ug[:, 1 : HP - 1, 0:2], 0.0)
            nc.vector.memset(ug[:, 1 : HP - 1, WP - 2 : WP], 0.0)

            # preload ACT function tables off the critical path
            dummy = small.tile([C, 2], F32)
            nc.vector.memset(dummy, 0.0)
            dump = small.tile([C, 2], F32)
            for fn in (AF.Silu, AF.Sigmoid):
                nc.scalar.activation(dump, dummy, fn, bias=0.0, scale=1.0)

            with tc.tile_pool(name="psA", bufs=1, space="PSUM") as psA:
                ps_yh = psA.tile([MIP, 64], F32, tag="yh")
                ps_ah = psA.tile([C, H], F32, tag="ah")
                ps_aw = psA.tile([C, W], F32, tag="aw")
                ah_sb = small.tile([C, H], BF16)

                def emit_tree(r0, nr):
                    # 32-col segment sums for rows [r0, r0+nr) (rows >= 64)
                    eng = nc.vector
                    xc = xs[:, ds(r0 * W, nr * W)].rearrange(
                        "p (y q s) -> p y q s", q=4, s=32
                    )
                    t1 = pp.tile([C, 16, 4, 16], BF16, tag="t1")
                    eng.tensor_add(t1[:, :nr], xc[:, :, :, 0:16], xc[:, :, :, 16:32])
                    t2 = pp.tile([C, 16, 4, 8], BF16, tag="t2")
                    eng.tensor_add(t2[:, :nr], t1[:, :nr, :, 0:8], t1[:, :nr, :, 8:16])
                    t3 = pp.tile([C, 16, 4, 4], BF16, tag="t3")
                    eng.tensor_add(t3[:, :nr], t2[:, :nr, :, 0:4], t2[:, :nr, :, 4:8])
                    t4 = pp.tile([C, 16, 4, 2], BF16, tag="t4")
                    eng.tensor_add(t4[:, :nr], t3[:, :nr, :, 0:2], t3[:, :nr, :, 2:4])
                    eng.tensor_add(
                        s32b[:, ds(r0 - 64, nr), :], t4[:, :nr, :, 0], t4[:, :nr, :, 1]
                    )

                def bn_hswish(src, dst, n):
                    # dst = h_swish(s1*src + t1f) for an (MIP, n) slice
                    z6 = pp.tile([MIP, n], F32, tag="bn_z6")
                    nc.vector.tensor_scalar(
                        out=z6, in0=src, scalar1=p8_sb[:, 0:1],
                        scalar2=p8_sb[:, 1:2], op0=ALU.mult, op1=ALU.add,
                    )
                    r = pp.tile([MIP, n], F32, tag="bn_r")
                    nc.vector.tensor_scalar(
                        out=r, in0=z6, scalar1=6.0, scalar2=3.0,
                        op0=ALU.mult, op1=ALU.add,
                    )
                    rc = pp.tile([MIP, n], F32, tag="bn_rc")
                    nc.vector.tensor_scalar(
                        out=rc, in0=r, scalar1=0.0, scalar2=6.0,
                        op0=ALU.max, op1=ALU.min,
                    )
                    nc.vector.tensor_mul(dst, z6, rc)

                def ah_mm(rlo, rhi, xh_sh):
                    nc.tensor.matmul(
                        ps_ah[:, ds(rlo, rhi - rlo)], wht_sb, xh_sh,
                        start=True, stop=True,
                    )
                    nc.scalar.activation(
                        ah_sb[:, ds(rlo, rhi - rlo)], ps_ah[:, ds(rlo, rhi - rlo)],
                        AF.Sigmoid, bias=p128_sb[:, 0:1], scale=1.0,
                    )

                def gate_rows(rlo, rhi):
                    # ug rows = x * a_h[c,y] * a_w[c,x], fused per row
                    for y in range(rlo, rhi):
                        nc.vector.scalar_tensor_tensor(
                            out=ug[:, 1 + y, 2 : 2 + W],
                            in0=xs[:, ds(y * W, W)],
                            scalar=ah_sb[:, y : y + 1],
                            in1=aw_sb,
                            op0=ALU.mult,
                            op1=ALU.mult,
                        )

                with tc.tile_pool(name="psW", bufs=1, space="PSUM") as psW:
                    ps_warm = psW.tile([C, 512], F32, tag="warm")
                    # x_w row-pool: 4-row matmuls with range-prescaled w1
                    # accumulate onto ONE (8, 4, W) psum bank
                    ps_xw = psW.tile([MIP, 4, W], F32, tag="xw")
                    # x_h col-pool for rows 0-63: per 16-row group, 8
                    # 16-col seg-matmuls (weights c1/c1/c2/c2/c3.. per seg)
                    # accumulate w1-projected partial sums
                    ps_xh0 = psW.tile([MIP, 64, 16], F32, tag="xh0")

                    def warm(n):
                        # dep-free warms: scheduled whenever PE is idle early
                        for _ in range(n):
                            nc.tensor.matmul(
                                ps_warm, wtile[:, 0:C], wtile,
                                start=True, stop=True,
                            )

                    def warm_tail(n):
                        # warms pinned on the LAST x chunk: fill the PE gap
                        # between the pool matmuls and the conv
                        for _ in range(n):
                            nc.tensor.matmul(
                                ps_warm, wtile[:, 0:C],
                                xs[:, ds(H * W - 512, 512)],
                                start=True, stop=True,
                            )

                    # ---- PE ramp-up: ~3.5us of warms before chunk 0 lands
                    warm(9)

                    # ---- chunk-chasing ----
                    for r0, nr in XCH:
                        for g4 in range(r0, r0 + nr, 4):
                            nc.tensor.matmul(
                                ps_xw,
                                w1ts_sb[:, min(g4 // 32, 2), :],
                                xs[:, ds(g4 * W, 4 * W)],
                                start=(g4 == 0),
                                stop=(g4 == H - 4),
                                skip_group_check=True,
                            )
                        if r0 < 64:
                            # x_h seg-matmuls, 16-row psum slices
                            for b0 in range(r0, r0 + nr, 16):
                                nb = min(16, r0 + nr - b0)
                                xv = xs[:, ds(b0 * W, nb * W)].rearrange(
                                    "p (y q s) -> p y q s", q=8, s=16
                                )
                                for s in range(8):
                                    nc.tensor.matmul(
                                        ps_xh0[:, ds(b0, nb), :],
                                        w1ts_sb[:, min(s // 2, 2), :],
                                        xv[:, :, s, :],
                                        start=(s == 0),
                                        stop=(s == 7),
                                        skip_group_check=True,
                                    )
                        elif r0 % 16 == 0 and nr >= 16:
                            for t0 in range(r0, r0 + nr, 16):
                                emit_tree(t0, 16)
                        elif r0 % 16 == 8:
                            emit_tree(r0 - 8, 16)
                        if r0 + nr == 64:
                            # rows 0-63 x_h: reduce psum -> yh directly (w1
                            # already applied), then bn + a_h matmul+sigmoid
                            yh0 = small.tile([MIP, 64], F32)
                            for hh in range(2):
                                nc.vector.tensor_reduce(
                                    out=yh0[:, ds(32 * hh, 32)],
                                    in_=ps_xh0[:, ds(32 * hh, 32), :],
                                    axis=mybir.AxisListType.X,
                                    op=ALU.add,
                                )
                            xhsA = pp.tile([MIP, 64], BF16, tag="xh_sh0")
                            bn_hswish(yh0, xhsA, 64)
                            ah_mm(0, 64, xhsA)

                    # ---- a_w chain: the critical path ----
                    xwp = small.tile([MIP, W], F32)
                    nc.vector.tensor_reduce(
                        out=xwp,
                        in_=ps_xw.rearrange("p j x -> p x j"),
                        axis=mybir.AxisListType.X,
                        op=ALU.add,
                    )
                    xw_s = small.tile([MIP, W], BF16)
                    bn_hswish(xwp, xw_s, W)
                    nc.tensor.matmul(ps_aw, wwt_sb, xw_s, start=True, stop=True)
                    aw_sb = small.tile([C, W], BF16)
                    nc.scalar.activation(
                        aw_sb, ps_aw, AF.Sigmoid, bias=p128_sb[:, 1:2], scale=1.0
                    )

                    # rows 64-127 x_h path (combines the DVE trees): pooled
                    # -> yh matmul -> bn -> a_h matmul+sigmoid, all BEFORE
                    # the conv's SiLUs start (one ACT table switch total)
                    # xhpB uses integer weights 7/3/1; the stationary
                    # w1ts[2] = w1/128 then yields exactly
                    # w1 @ (7*S0 + 3*S1 + S2 + S3)/128 = w1 @ x_h-pool
                    tmpA = pp.tile([C, 64], F32, tag="tmpA")
                    nc.vector.tensor_add(tmpA, s32b[:, :, 2], s32b[:, :, 3])
                    m0 = pp.tile([C, 64], F32, tag="m0")
                    nc.vector.tensor_scalar_mul(m0, s32b[:, :, 0], 7.0)
                    m1 = pp.tile([C, 64], F32, tag="m1")
                    nc.vector.scalar_tensor_tensor(
                        out=m1, in0=s32b[:, :, 1], scalar=3.0, in1=m0,
                        op0=ALU.mult, op1=ALU.add,
                    )
                    xhpB = pp.tile([C, 64], BF16, tag="xhpB")
                    nc.vector.tensor_add(xhpB, tmpA, m1)
                    nc.tensor.matmul(
                        ps_yh, w1ts_sb[:, 2, :], xhpB, start=True, stop=True
                    )
                    xhsB = pp.tile([MIP, 64], BF16, tag="xh_sh64")
                    bn_hswish(ps_yh, xhsB, 64)
                    ah_mm(64, 128, xhsB)

                    # bridge the post-pool PE idle until the conv starts
                    warm_tail(20)
                    # small warms pinned on aw_sb cover the sigmoid->gate
                    # window without delaying the attention matmuls above
                    for _ in range(12):
                        nc.tensor.matmul(
                            ps_warm[:, 0:W], wct_sb[:, 0, :], aw_sb,
                            start=True, stop=True,
                        )

                # gate rows; the conv chases these
                gate_rows(0, 64)
                gate_rows(64, 128)

                # ---- 3x3 conv + BN2 + SiLU ----
                with (
                    tc.tile_pool(name="psB", bufs=5, space="PSUM") as psB,
                    tc.tile_pool(name="obp", bufs=4) as obp,
                ):
                    for rb in range(H // 4):
                        pso = psB.tile([C, 4, W], F32, tag="pso")
                        for k in range(9):
                            dy, dx = k // 3, k % 3
                            nc.tensor.matmul(
                                pso,
                                wct_sb[:, k, :],
                                ug[:, 4 * rb + dy : 4 * rb + dy + 4,
                                   1 + dx : 1 + dx + W],
                                start=(k == 0),
                                stop=(k == 8),
                            )
                        ob = obp.tile([C, 4, W], BF16, tag="ob")
                        nc.scalar.activation(
                            ob, pso, AF.Silu,
                            bias=p128_sb[:, 3:4], scale=p128_sb[:, 2:3],
                        )
                        nc.sync.dma_start(
                            out=outp[:, 4 * rb : 4 * rb + 4, :], in_=ob
                        )

    nc.compile()
    return nc


def prep_inputs(x, w1, b1, g1, be1, m1, v1, wh, bh, ww, bw, wc, bc, g2, be2, m2, v2):
    """Host-side prep: per-core input maps (weights replicated)."""
    bf = ml_dtypes.bfloat16
    N = x.shape[0]
    s1 = (g1 / np.sqrt(v1 + EPS)).astype(np.float64)
    t1f = s1 * b1 + be1 - m1 * s1
    p8 = np.stack([s1 / 6.0, t1f / 6.0, s1, t1f + 3.0], axis=1).astype(np.float32)
    s2 = (g2 / np.sqrt(v2 + EPS)).astype(np.float64)
    b2 = bc * s2 + be2 - m2 * s2
    p128 = np.stack([bh, bw, s2, b2], axis=1).astype(np.float32)
    cc = np.array([7.0 / 128, 3.0 / 128, 1.0 / 128])
    w1ts = np.stack([w1.T * c for c in cc], axis=1)              # (C, 3, MIP)
    shared = {
        "w1ts": np.ascontiguousarray(w1ts.reshape(C, 3 * MIP)).astype(bf),
        "wht": np.ascontiguousarray(wh.T).astype(bf),            # (MIP, C)
        "wwt": np.ascontiguousarray(ww.T).astype(bf),            # (MIP, C)
        "wct": np.ascontiguousarray(
            np.transpose(wc, (1, 2, 3, 0)).reshape(C, 9 * C)
        ).astype(bf),                                            # [i, (ky kx), o]
        "p8": p8,
        "p128": p128,
    }
    in_maps = []
    for n in range(N):
        m = dict(shared)
        m["x"] = np.ascontiguousarray(x[n].reshape(C, H * W)).astype(bf)
        in_maps.append(m)
    return in_maps


def run(inputs, trace=False):
    if "nc" not in _CACHE:
        _CACHE["nc"] = build_nc()
    nc = _CACHE["nc"]
    in_maps = prep_inputs(**inputs)
    res = run_bass_kernel_spmd(nc, in_maps, core_ids=list(range(8)), trace=trace)
    out = np.stack([np.asarray(res.results[i]["out"]) for i in range(8)], axis=0)
    return out.astype(np.float32), res


def kernel(**inputs) -> np.ndarray:
    out, _ = run(inputs, trace=False)
    return out



# revision 2
# speedup vs baseline: 1.1543x; 1.1543x over previous
"""Trainium2 Bass kernel for CAConv2 (coordinate-attention + 3x3 conv block).

Shapes (hardcoded): x (8, 128, 128, 128) f32; data-parallel over batch,
one image per NeuronCore (8 cores).

Scheduling facts learned from HW traces / the cost model:
- PE streams moving data at ~1.97 GHz sustained (0.508 ns/col, bf16),
  even with identical stationary reloads -> the 3x3 conv (288 matmuls
  x 512 cols) is a hard ~75us floor; everything else must hide around it.
- Engines list-schedule with a 4-deep stalled-instruction bypass window:
  emission position is priority, dependencies are eligibility.
- DMA engines (16) sustain ~21.3 GB/s each (HBM roofline ~341 GB/s);
  x (4MB bf16) needs ~13us of drain. DMA triggers cost ~0.8us each on
  the issuing queue, so x triggers go first on the gpsimd SWDGE ring and
  wct queues behind x on the same in-order ring (no HBM contention).
- ACT tables load per *set*; Tanh and Silu share silu_and_others, so all
  sigmoids are computed as 0.5+0.5*tanh(z/2) (affine folded into small
  DVE ops) -> exactly one ACT_TABLE_LOAD for the whole kernel.
- Cross-engine dependency hops cost ~0.3-1us of semaphore latency.
"""

import numpy as np
import ml_dtypes

import concourse.bacc as bacc
import concourse.tile as tile
from concourse import mybir
from concourse.bass import ds
from concourse.bass_utils import run_bass_kernel_spmd

BF16 = mybir.dt.bfloat16
F32 = mybir.dt.float32
C, H, W, MIP = 128, 128, 128, 8
WP = W + 4  # padded width: cols [2, 130) hold data, 0/1 and 130/131 are zero
HP = H + 2  # padded height: rows [1, 129) hold data
EPS = 1e-5
AF = mybir.ActivationFunctionType
ALU = mybir.AluOpType

_CACHE = {}


def build_nc():
    nc = bacc.Bacc(num_swdge_queues=1)
    xp = nc.declare_dram_parameter("x", [C, H * W], BF16, isOutput=False)
    w1ts = nc.declare_dram_parameter("w1ts", [C, 3 * MIP], BF16, isOutput=False)
    wht = nc.declare_dram_parameter("wht", [MIP, C], BF16, isOutput=False)
    wwt = nc.declare_dram_parameter("wwt", [MIP, C], BF16, isOutput=False)
    # wct[i, k, o] = wc[o, i, k//3, k%3]
    wct = nc.declare_dram_parameter("wct", [C, 9 * C], BF16, isOutput=False)
    # p8 cols: 0: s1/6, 1: t1f/6, 2: s1, 3: t1f+3   (t1f = s1*b1 + be1 - m1*s1)
    p8 = nc.declare_dram_parameter("p8", [MIP, 4], F32, isOutput=False)
    # p128 cols: 0: bh/2, 1: bw/2, 2: s2, 3: b2 (= bc*s2 + be2 - m2*s2)
    p128 = nc.declare_dram_parameter("p128", [C, 4], F32, isOutput=False)
    outp = nc.declare_dram_parameter("out", [C, H, W], BF16, isOutput=True)

    with tile.TileContext(nc) as tc:
        with (
            tc.tile_pool(name="sing", bufs=1) as sing,
            tc.tile_pool(name="pp", bufs=2) as pp,
            tc.tile_pool(name="small", bufs=1) as small,
        ):
            xs = sing.tile([C, H * W], BF16)
            ug = sing.tile([C, HP, WP], BF16)
            s32b = sing.tile([C, 64, 4], F32)  # col-segment sums rows 64-127
            wtile = sing.tile([C, 512], BF16)  # zeros; warm-matmul fodder

            # x chunks go FIRST on the gpsimd SWDGE ring (earliest possible
            # stream start); descriptors drain in issue order at full BW.
            XCH = [(0, 16), (16, 16), (32, 32), (64, 32), (96, 16), (112, 8), (120, 8)]
            for r0, nr in XCH:
                nc.gpsimd.dma_start(
                    out=xs[:, ds(r0 * W, nr * W)],
                    in_=xp[:, ds(r0 * W, nr * W)],
                )
            # wct queues BEHIND x on the same in-order ring: its 288KB only
            # move after x fully drains (no HBM contention mid-stream), and
            # it still lands ~1us after x -- well before the conv needs it.
            wct_sb = sing.tile([C, 9, C], BF16)
            nc.gpsimd.dma_start(out=wct_sb, in_=wct.rearrange("i (k o) -> i k o", k=9))

            # small weights/params ride the sync ring; w1ts first (first matmuls)
            w1ts_sb = sing.tile([C, 3, MIP], BF16)
            nc.sync.dma_start(
                out=w1ts_sb, in_=w1ts.rearrange("c (r m) -> c r m", r=3)
            )
            wht_sb = sing.tile([MIP, C], BF16)
            nc.sync.dma_start(out=wht_sb, in_=wht[:, :])
            wwt_sb = sing.tile([MIP, C], BF16)
            nc.sync.dma_start(out=wwt_sb, in_=wwt[:, :])
            p8_sb = sing.tile([MIP, 4], F32)
            nc.sync.dma_start(out=p8_sb, in_=p8[:, :])
            p128_sb = sing.tile([C, 4], F32)
            nc.sync.dma_start(out=p128_sb, in_=p128[:, :])

            # warm fodder first so the PE can start ramping ASAP
            nc.vector.memset(wtile, 0.0)
            # conv padding border of ug
            nc.vector.memset(ug[:, 0, :], 0.0)
            nc.vector.memset(ug[:, HP - 1, :], 0.0)
            nc.vector.memset(ug[:, 1 : HP - 1, 0:2], 0.0)
            nc.vector.memset(ug[:, 1 : HP - 1, WP - 2 : WP], 0.0)

            # one ACT table load for the whole kernel: silu_and_others also
            # contains Tanh, which serves every sigmoid via 0.5+0.5*tanh(z/2)
            dump = small.tile([C, 2], F32)
            nc.scalar.activation(dump, wtile[:, 0:2], AF.Silu, bias=0.0, scale=1.0)

            with tc.tile_pool(name="psA", bufs=1, space="PSUM") as psA:
                ps_yh = psA.tile([MIP, 64], F32, tag="yh")
                ps_ah = psA.tile([C, H], F32, tag="ah")
                ps_aw = psA.tile([C, W], F32, tag="aw")
                ah_sb = small.tile([C, H], BF16)
                ah_t = small.tile([C, H], F32)  # raw tanh before affine

                def emit_tree(r0, nr):
                    # 32-col segment sums for rows [r0, r0+nr) (rows >= 64)
                    eng = nc.vector
                    xc = xs[:, ds(r0 * W, nr * W)].rearrange(
                        "p (y q s) -> p y q s", q=4, s=32
                    )
                    t1 = pp.tile([C, 16, 4, 16], BF16, tag="t1")
                    eng.tensor_add(t1[:, :nr], xc[:, :, :, 0:16], xc[:, :, :, 16:32])
                    t2 = pp.tile([C, 16, 4, 8], BF16, tag="t2")
                    eng.tensor_add(t2[:, :nr], t1[:, :nr, :, 0:8], t1[:, :nr, :, 8:16])
                    t3 = pp.tile([C, 16, 4, 4], BF16, tag="t3")
                    eng.tensor_add(t3[:, :nr], t2[:, :nr, :, 0:4], t2[:, :nr, :, 4:8])
                    t4 = pp.tile([C, 16, 4, 2], BF16, tag="t4")
                    eng.tensor_add(t4[:, :nr], t3[:, :nr, :, 0:2], t3[:, :nr, :, 2:4])
                    eng.tensor_add(
                        s32b[:, ds(r0 - 64, nr), :], t4[:, :nr, :, 0], t4[:, :nr, :, 1]
                    )

                def bn_hswish(src, dst, n):
                    # dst = h_swish(s1*src + t1f) for an (MIP, n) slice
                    z6 = pp.tile([MIP, n], F32, tag="bn_z6")
                    nc.vector.tensor_scalar(
                        out=z6, in0=src, scalar1=p8_sb[:, 0:1],
                        scalar2=p8_sb[:, 1:2], op0=ALU.mult, op1=ALU.add,
                    )
                    r = pp.tile([MIP, n], F32, tag="bn_r")
                    nc.vector.tensor_scalar(
                        out=r, in0=z6, scalar1=6.0, scalar2=3.0,
                        op0=ALU.mult, op1=ALU.add,
                    )
                    rc = pp.tile([MIP, n], F32, tag="bn_rc")
                    nc.vector.tensor_scalar(
                        out=rc, in0=r, scalar1=0.0, scalar2=6.0,
                        op0=ALU.max, op1=ALU.min,
                    )
                    nc.vector.tensor_mul(dst, z6, rc)

                def ah_mm(rlo, rhi, xh_sh):
                    # a_h = sigmoid(wh @ xh + bh) via tanh: 0.5+0.5*tanh(.5z+.5bh)
                    nc.tensor.matmul(
                        ps_ah[:, ds(rlo, rhi - rlo)], wht_sb, xh_sh,
                        start=True, stop=True,
                    )
                    nc.scalar.activation(
                        ah_t[:, ds(rlo, rhi - rlo)], ps_ah[:, ds(rlo, rhi - rlo)],
                        AF.Tanh, bias=p128_sb[:, 0:1], scale=0.5,
                    )
                    nc.vector.tensor_scalar(
                        out=ah_sb[:, ds(rlo, rhi - rlo)],
                        in0=ah_t[:, ds(rlo, rhi - rlo)],
                        scalar1=0.5, scalar2=0.5, op0=ALU.mult, op1=ALU.add,
                    )

                def gate_rows(rlo, rhi):
                    # ug rows = x * a_h[c,y] * a_w[c,x], fused per row
                    for y in range(rlo, rhi):
                        nc.vector.scalar_tensor_tensor(
                            out=ug[:, 1 + y, 2 : 2 + W],
                            in0=xs[:, ds(y * W, W)],
                            scalar=ah_sb[:, y : y + 1],
                            in1=aw_sb,
                            op0=ALU.mult,
                            op1=ALU.mult,
                        )

                with tc.tile_pool(name="psW", bufs=1, space="PSUM") as psW:
                    ps_warm = psW.tile([C, 512], F32, tag="warm")
                    # x_w row-pool: 4-row matmuls with range-prescaled w1
                    # accumulate onto ONE (8, 4, W) psum bank
                    ps_xw = psW.tile([MIP, 4, W], F32, tag="xw")
                    # x_h col-pool for rows 0-63: per 16-row group, 8
                    # 16-col seg-matmuls (weights c1/c1/c2/c2/c3.. per seg)
                    # accumulate w1-projected partial sums
                    ps_xh0 = psW.tile([MIP, 64, 16], F32, tag="xh0")

                    def warm(n):
                        # dep-free warms: bridge PE gaps / hold the p-state
                        for _ in range(n):
                            nc.tensor.matmul(
                                ps_warm, wtile[:, 0:C], wtile,
                                start=True, stop=True,
                            )

                    def warm_tail(n):
                        # warms pinned on the LAST x chunk: fill the PE gap
                        # between the pool matmuls and the conv
                        for _ in range(n):
                            nc.tensor.matmul(
                                ps_warm, wtile[:, 0:C],
                                xs[:, ds(H * W - 512, 512)],
                                start=True, stop=True,
                            )

                    # ---- short PE ramp before chunk 0 lands
                    warm(2)

                    # ---- chunk-chasing ----
                    for r0, nr in XCH:
                        for g4 in range(r0, r0 + nr, 4):
                            nc.tensor.matmul(
                                ps_xw,
                                w1ts_sb[:, min(g4 // 32, 2), :],
                                xs[:, ds(g4 * W, 4 * W)],
                                start=(g4 == 0),
                                stop=(g4 == H - 4),
                                skip_group_check=True,
                            )
                        if r0 < 64:
                            # x_h seg-matmuls, 16-row psum slices
                            for b0 in range(r0, r0 + nr, 16):
                                nb = min(16, r0 + nr - b0)
                                xv = xs[:, ds(b0 * W, nb * W)].rearrange(
                                    "p (y q s) -> p y q s", q=8, s=16
                                )
                                for s in range(8):
                                    nc.tensor.matmul(
                                        ps_xh0[:, ds(b0, nb), :],
                                        w1ts_sb[:, min(s // 2, 2), :],
                                        xv[:, :, s, :],
                                        start=(s == 0),
                                        stop=(s == 7),
                                        skip_group_check=True,
                                    )
                        elif r0 % 16 == 0 and nr >= 16 and r0 + nr <= 112:
                            # trees for rows 64-111 fill DVE idle mid-stream;
                            # tree 112-127 is deferred (would congest the
                            # vector queue right when the a_w chain must run)
                            for t0 in range(r0, r0 + nr, 16):
                                emit_tree(t0, 16)
                        if r0 + nr == 64:
                            # rows 0-63 x_h: reduce psum -> yh directly (w1
                            # already applied), then bn + a_h matmul+tanh
                            yh0 = small.tile([MIP, 64], F32)
                            for hh in range(2):
                                nc.vector.tensor_reduce(
                                    out=yh0[:, ds(32 * hh, 32)],
                                    in_=ps_xh0[:, ds(32 * hh, 32), :],
                                    axis=mybir.AxisListType.X,
                                    op=ALU.add,
                                )
                            xhsA = pp.tile([MIP, 64], BF16, tag="xh_sh0")
                            bn_hswish(yh0, xhsA, 64)
                            ah_mm(0, 64, xhsA)

                    # ---- a_w chain: the critical path ----
                    xwp = small.tile([MIP, W], F32)
                    nc.vector.tensor_reduce(
                        out=xwp,
                        in_=ps_xw.rearrange("p j x -> p x j"),
                        axis=mybir.AxisListType.X,
                        op=ALU.add,
                    )
                    xw_s = small.tile([MIP, W], BF16)
                    bn_hswish(xwp, xw_s, W)
                    nc.tensor.matmul(ps_aw, wwt_sb, xw_s, start=True, stop=True)
                    aw_t = small.tile([C, W], F32)
                    nc.scalar.activation(
                        aw_t, ps_aw, AF.Tanh, bias=p128_sb[:, 1:2], scale=0.5
                    )
                    aw_sb = small.tile([C, W], BF16)
                    nc.vector.tensor_scalar(
                        out=aw_sb, in0=aw_t,
                        scalar1=0.5, scalar2=0.5, op0=ALU.mult, op1=ALU.add,
                    )

                    # first gates unblock conv block 0 ASAP
                    gate_rows(0, 16)

                    # rows 64-127 x_h path (deferred tree + combine): pooled
                    # -> yh matmul -> bn -> a_h matmul+tanh. Fully off the
                    # critical path (conv reaches row 64 ~35us later).
                    # xhpB uses integer weights 7/3/1; the stationary
                    # w1ts[2] = w1/128 then yields exactly
                    # w1 @ (7*S0 + 3*S1 + S2 + S3)/128 = w1 @ x_h-pool
                    emit_tree(112, 16)
                    tmpA = pp.tile([C, 64], F32, tag="tmpA")
                    nc.vector.tensor_add(tmpA, s32b[:, :, 2], s32b[:, :, 3])
                    m0 = pp.tile([C, 64], F32, tag="m0")
                    nc.vector.tensor_scalar_mul(m0, s32b[:, :, 0], 7.0)
                    m1 = pp.tile([C, 64], F32, tag="m1")
                    nc.vector.scalar_tensor_tensor(
                        out=m1, in0=s32b[:, :, 1], scalar=3.0, in1=m0,
                        op0=ALU.mult, op1=ALU.add,
                    )
                    xhpB = pp.tile([C, 64], BF16, tag="xhpB")
                    nc.vector.tensor_add(xhpB, tmpA, m1)
                    nc.tensor.matmul(
                        ps_yh, w1ts_sb[:, 2, :], xhpB, start=True, stop=True
                    )
                    xhsB = pp.tile([MIP, 64], BF16, tag="xh_sh64")
                    bn_hswish(ps_yh, xhsB, 64)
                    ah_mm(64, 128, xhsB)

                    # bridge the post-pool PE idle until the conv starts
                    warm_tail(17)

                # remaining gates; the conv chases these
                gate_rows(16, 64)
                gate_rows(64, 128)

                # ---- 3x3 conv + BN2 + SiLU ----
                with (
                    tc.tile_pool(name="psB", bufs=5, space="PSUM") as psB,
                    tc.tile_pool(name="obp", bufs=4) as obp,
                ):
                    for rb in range(H // 4):
                        pso = psB.tile([C, 4, W], F32, tag="pso")
                        for k in range(9):
                            dy, dx = k // 3, k % 3
                            nc.tensor.matmul(
                                pso,
                                wct_sb[:, k, :],
                                ug[:, 4 * rb + dy : 4 * rb + dy + 4,
                                   1 + dx : 1 + dx + W],
                                start=(k == 0),
                                stop=(k == 8),
                            )
                        ob = obp.tile([C, 4, W], BF16, tag="ob")
                        nc.scalar.activation(
                            ob, pso, AF.Silu,
                            bias=p128_sb[:, 3:4], scale=p128_sb[:, 2:3],
                        )
                        nc.sync.dma_start(
                            out=outp[:, 4 * rb : 4 * rb + 4, :], in_=ob
                        )

    nc.compile()
    return nc


def prep_inputs(x, w1, b1, g1, be1, m1, v1, wh, bh, ww, bw, wc, bc, g2, be2, m2, v2):
    """Host-side prep: per-core input maps (weights replicated)."""
    bf = ml_dtypes.bfloat16
    N = x.shape[0]
    s1 = (g1 / np.sqrt(v1 + EPS)).astype(np.float64)
    t1f = s1 * b1 + be1 - m1 * s1
    p8 = np.stack([s1 / 6.0, t1f / 6.0, s1, t1f + 3.0], axis=1).astype(np.float32)
    s2 = (g2 / np.sqrt(v2 + EPS)).astype(np.float64)
    b2 = bc * s2 + be2 - m2 * s2
    # bh/bw pre-halved for the tanh-based sigmoid
    p128 = np.stack([0.5 * bh, 0.5 * bw, s2, b2], axis=1).astype(np.float32)
    cc = np.array([7.0 / 128, 3.0 / 128, 1.0 / 128])
    w1ts = np.stack([w1.T * c for c in cc], axis=1)              # (C, 3, MIP)
    shared = {
        "w1ts": np.ascontiguousarray(w1ts.reshape(C, 3 * MIP)).astype(bf),
        "wht": np.ascontiguousarray(wh.T).astype(bf),            # (MIP, C)
        "wwt": np.ascontiguousarray(ww.T).astype(bf),            # (MIP, C)
        "wct": np.ascontiguousarray(
            np.transpose(wc, (1, 2, 3, 0)).reshape(C, 9 * C)
        ).astype(bf),                                            # [i, (ky kx), o]
        "p8": p8,
        "p128": p128,
    }
    in_maps = []
    for n in range(N):
        m = dict(shared)
        m["x"] = np.ascontiguousarray(x[n].reshape(C, H * W)).astype(bf)
        in_maps.append(m)
    return in_maps


def run(inputs, trace=False):
    if "nc" not in _CACHE:
        _CACHE["nc"] = build_nc()
    nc = _CACHE["nc"]
    in_maps = prep_inputs(**inputs)
    res = run_bass_kernel_spmd(nc, in_maps, core_ids=list(range(8)), trace=trace)
    out = np.stack([np.asarray(res.results[i]["out"]) for i in range(8)], axis=0)
    return out.astype(np.float32), res


def kernel(**inputs) -> np.ndarray:
    out, _ = run(inputs, trace=False)
    return out
